# revision 23
# baseline (speedup 1.0000x reference)
"""Trainium2 Bass kernel for nn_BRCLoss (supervised-contrastive style loss).

Math (per batch sample b, matching the jax reference):
    f = features[b].reshape(24, 4096); fhat = f / ||f||_row
    logits = (fhat @ fhat.T) / 0.1                       # [24, 24]
    exp_logits = exp(logits) * (1 - I)
    log_prob = logits - log(exp_logits.sum(-1))
    mlpp = (mask * log_prob).sum(-1) / (mask.sum(-1) + 1e-6)
    loss = sum_b mean_m(-0.1 * mlpp) / 512               # scalar

`outputs` / `targets` are unused by the reference; only `features`
[512, 2, 12, 4096] f32 matters.  Pure data parallel: 64 samples per core,
per-core partial sums added on the host.

Per-core kernel (~90 us, HBM-read roofline ~70 us at ~360 GB/s):
  - 13 tiles of [120 rows, 4096] (5 samples each; the last tile re-reads the
    trailing 120 rows and the duplicated sample is zero-weighted via W1/W2).
  - Feature loads are SWDGE (gpsimd) DMAs that cast f32 -> bf16 in flight:
    HBM still reads the full f32 stream (the memory roofline), but SBUF
    writes halve, which relieves the port bottleneck shared with the
    sibling NeuronCore under 8-core SPMD.
  - Per tile: PE-transposes 32 bf16 chunks [120,128] -> PSUM (4 per bank),
    DVE/ACT copies them to SBUF, then 32 accumulating [128,120]^T[128,120]
    bf16 matmuls build the raw block-diagonal Gram G [120,120] (f32 PSUM).
  - Per-tile epilogue: d2 = 0.1*diag(G) via a pre-scaled identity mask;
    rnx = d2^-0.5 computed as exp(-0.5*ln(d2)); logits via
    L = diag(rnx) @ G @ diag(rnx), where the column scale runs on the PE as
    G @ diag(rnx) (G symmetric, so lhsT=G) and the row scale as a
    per-partition tensor_scalar; then exp, masked reductions, and ln of the
    denominator.  Host-precomputed masks/weights (m0/pm/w1/w2) fold the
    self-exclusion, positives graph, positive counts + eps, anchor mean,
    1/B and tile-12 deduplication into one final weighted dot product.
  - All activations (Ln/Exp/Copy) are pinned to the single
    natural_log_exp_and_others ACT table set (see _OneActSetBacc) so the
    kernel pays exactly one ACT_TABLE_LOAD instead of ~1.3 us per Ln<->Exp
    alternation.
"""

import os
import sys

import numpy as np

if "/opt/trn_rl_repo" not in sys.path:
    sys.path.insert(0, "/opt/trn_rl_repo")

# Problem constants (hardcoded; kernel.py must be self-contained).
B = 512
NV = 2
NCLS = 12
D = 4096
M = NV * NCLS              # 24 anchor rows per sample
NCORES = 8
SPC = B // NCORES          # 64 samples per core
ROWS = SPC * M             # 1536 feature rows per core
P = 120                    # rows per tile (5 samples)
G5 = P // M                # samples per tile
T = 13                     # tiles per core (12 aligned + 1 overlapping tail)
CH = 128                   # contraction chunk (PE partition limit)
NCH = D // CH              # 32 chunks
QUAD = 4                   # transposed chunks packed per PSUM bank
NQ = NCH // QUAD
TEMP = 0.1
EPS_POS = 1e-6

_ROW_STARTS = [P * t for t in range(T - 1)] + [ROWS - P]

_compiled = None           # (nc, const_in_map)
LAST_RESULTS = None        # BassKernelResults of the most recent run


def _host_consts():
    """Masks/weights shared by every core (the per-core sample grid is identical)."""
    i = np.arange(NCLS)
    graph = (np.abs(i[:, None] - i[None, :]) <= 1).astype(np.float32)   # [12,12]
    eye24 = np.eye(M, dtype=np.float32)
    mask24 = np.tile(graph, (NV, NV)) * (1.0 - eye24)                    # positives
    blk = np.kron(np.eye(G5, dtype=np.float32), np.ones((M, M), np.float32))
    m0 = (blk * (1.0 - np.eye(P, dtype=np.float32))).astype(np.float32)  # denom mask
    pm = np.kron(np.eye(G5, dtype=np.float32), mask24).astype(np.float32)
    im = (TEMP * np.eye(P)).astype(np.float32)   # folds the 1/temperature scale
    msum = np.tile(mask24.sum(1), G5).astype(np.float64)                 # [120], 3 or 5
    alpha = -TEMP / ((msum + EPS_POS) * M * B)                           # per-row weight
    valid = np.ones((P, T), np.float64)
    valid[:M, T - 1] = 0.0   # last tile re-reads sample 59 -> zero weight
    w1 = (alpha[:, None] * valid).astype(np.float32)
    w2 = ((-alpha * msum)[:, None] * valid).astype(np.float32)
    return {"m0": m0, "pm": pm, "im": im, "w1": w1, "w2": w2}


def _build():
    from contextlib import ExitStack

    from concourse import bacc, bass, masks, mybir, tile

    f32 = mybir.dt.float32
    bf16 = mybir.dt.bfloat16
    AX = mybir.AxisListType
    ALU = mybir.AluOpType
    ACTF = mybir.ActivationFunctionType

    import bass_rust as _bass_rust
    from concourse.hw_specs import get_activation_tables

    class _OneActSetBacc(bacc.Bacc):
        """Every activation used here (Ln, Exp, Copy) lives in the
        natural_log_exp_and_others ACT table set; restricting the set-choice
        pass to it means one ACT_TABLE_LOAD for the whole kernel instead of
        ~1.3us switches between the exp- and ln-anchored sets per use.  Other
        sets keep their list position (ids are positional) but advertise no
        functions, so the pass cannot pick them."""

        def insert_act_table_loads(self):
            has_activation = any(
                isinstance(i, mybir.InstActivation)
                for b in self.main_func.blocks
                for i in b.instructions
            )
            if not has_activation:
                return
            tables = [
                (n, (s if n == "natural_log_exp_and_others" else set()))
                for n, s in get_activation_tables(self.m.arch).items()
            ]
            _bass_rust.insert_act_table_loads(self, tables)

    nc = _OneActSetBacc("TRN2", target_bir_lowering=False, debug=False,
                        num_devices=NCORES)

    f_dram = nc.dram_tensor("f", (ROWS, D), f32, kind="ExternalInput")
    m0_dram = nc.dram_tensor("m0", (P, P), f32, kind="ExternalInput")
    pm_dram = nc.dram_tensor("pm", (P, P), f32, kind="ExternalInput")
    im_dram = nc.dram_tensor("im", (P, P), f32, kind="ExternalInput")
    w1_dram = nc.dram_tensor("w1", (P, T), f32, kind="ExternalInput")
    w2_dram = nc.dram_tensor("w2", (P, T), f32, kind="ExternalInput")
    out_dram = nc.dram_tensor("out", (1, 1), f32, kind="ExternalOutput")

    DSPLIT = 2                 # DMAs per feature tile
    DCOLS = D // DSPLIT

    with ExitStack() as ctx:
        tc = ctx.enter_context(tile.TileContext(nc))
        consts = ctx.enter_context(tc.tile_pool(name="consts", bufs=1))
        fpool = ctx.enter_context(tc.tile_pool(name="fpool", bufs=8))
        tcpool = ctx.enter_context(tc.tile_pool(name="tcpool", bufs=6))
        work = ctx.enter_context(tc.tile_pool(name="work", bufs=1))
        lwork = ctx.enter_context(tc.tile_pool(name="lwork", bufs=2))
        small = ctx.enter_context(tc.tile_pool(name="small", bufs=2))
        tpsum = ctx.enter_context(
            tc.tile_pool(name="tpsum", bufs=3, space=bass.MemorySpace.PSUM))
        gpsum = ctx.enter_context(
            tc.tile_pool(name="gpsum", bufs=2, space=bass.MemorySpace.PSUM))
        rpsum = ctx.enter_context(
            tc.tile_pool(name="rpsum", bufs=2, space=bass.MemorySpace.PSUM))

        # Feature loads lead the program so the sync-ring FIFO starts streaming
        # them immediately; constants ride the scalar HWDGE ring in parallel.
        def load_tile(ft, t):
            r0 = _ROW_STARTS[t]
            nsp = 8 if t == T - 1 else (4 if t == T - 2 else DSPLIT)   # finer tail chunks
            w = D // nsp
            for q in range(nsp):
                nc.gpsimd.dma_start(ft[:, q * w:(q + 1) * w],
                                    f_dram[r0:r0 + P, q * w:(q + 1) * w])

        ftiles = []
        for t in range(T):
            ft = fpool.tile([P, D], bf16, tag="f")
            if t < 3:
                load_tile(ft, t)
            ftiles.append(ft)

        identb = consts.tile([128, 128], bf16, tag="identb")
        masks.make_identity(nc, identb[:])
        m0_t = consts.tile([P, P], f32, tag="m0")
        pm_t = consts.tile([P, P], f32, tag="pm")
        im_t = consts.tile([P, P], f32, tag="im")
        w1_t = consts.tile([P, T], f32, tag="w1")
        w2_t = consts.tile([P, T], f32, tag="w2")
        nc.scalar.dma_start(m0_t[:], m0_dram[:, :])
        nc.scalar.dma_start(pm_t[:], pm_dram[:, :])
        nc.scalar.dma_start(im_t[:], im_dram[:, :])
        nc.scalar.dma_start(w1_t[:], w1_dram[:, :])
        nc.scalar.dma_start(w2_t[:], w2_dram[:, :])

        # Preload the exp/ln activation table set while DMA streams.
        warm = consts.tile([1, 2], f32, tag="warm")
        nc.vector.memset(warm[:], 1.0)
        nc.scalar.activation(warm[:, 1:2], warm[:, 0:1], ACTF.Exp)

        t1cols = work.tile([P, T], f32, tag="t1cols")   # sum(mask*logits) per tile
        ldcols = work.tile([P, T], f32, tag="ldcols")   # log softmax denominators
        egpool = ctx.enter_context(tc.tile_pool(name="egpool", bufs=4))
        egs = {}

        def tile_gram(t):
            ft = ftiles[t]
            if t >= 3:
                load_tile(ft, t)
            g = gpsum.tile([P, P], f32, tag="g")
            for q in range(NQ):
                tp = tpsum.tile([128, QUAD * P], bf16, tag="tp")
                for j in range(QUAD):
                    c = q * QUAD + j
                    nc.tensor.transpose(
                        tp[:, j * P:(j + 1) * P],
                        ft[:, c * CH:(c + 1) * CH],
                        identb[:P, :P],
                    )
                tcs = tcpool.tile([128, QUAD * P], bf16, tag="tc")
                if q % 2 == 0:
                    nc.vector.tensor_copy(tcs[:], tp[:])
                else:
                    nc.scalar.copy(tcs[:], tp[:])
                for j in range(QUAD):
                    c = q * QUAD + j
                    sl = tcs[:, j * P:(j + 1) * P]
                    nc.tensor.matmul(g[:], sl, sl,
                                     start=(c == 0), stop=(c == NCH - 1))
            eg = egpool.tile([P, P], bf16, tag="eg")
            nc.vector.tensor_copy(eg[:], g[:])
            egs[t] = eg
            # d2 = 0.1 * diag(G)  (im_t is pre-scaled by TEMP)
            scr = lwork.tile([P, P], f32, tag="scr")
            nc.vector.tensor_tensor(scr[:], g[:], im_t[:], ALU.mult)
            d2 = small.tile([P, 1], f32, tag="d2")
            nc.vector.tensor_reduce(d2[:], scr[:], axis=AX.X, op=ALU.add)
            return d2

        def tile_softmax(t, d2):
            # rnx = (0.1*d2)^-0.5 via exp/ln (same ACT table set);
            # logits L = diag(rnx) @ G @ diag(rnx); the column scaling runs on
            # the PE as G @ diag(rnx) (G is symmetric so lhsT=G is G^T), the
            # row scaling as a per-partition tensor_scalar.
            eg = egs.pop(t)
            lnv = small.tile([P, 1], f32, tag="lnv")
            nc.scalar.activation(lnv[:], d2[:], ACTF.Ln)
            rnx = small.tile([P, 1], f32, tag="rnx")
            nc.scalar.activation(rnx[:], lnv[:], ACTF.Exp, scale=-0.5)
            drn = lwork.tile([P, P], bf16, tag="drn")
            nc.vector.tensor_scalar(drn[:], im_t[:], rnx[:], 1.0 / TEMP,
                                    op0=ALU.mult, op1=ALU.mult)
            h_ps = rpsum.tile([P, P], f32, tag="r")
            nc.tensor.matmul(h_ps[:], eg[:], drn[:], start=True, stop=True)
            lt = lwork.tile([P, P], f32, tag="lt")
            nc.vector.tensor_scalar_mul(lt[:], h_ps[:], rnx[:])
            xt = lwork.tile([P, P], f32, tag="xt")
            nc.scalar.activation(xt[:], lt[:], ACTF.Exp)
            xm = lwork.tile([P, P], f32, tag="xm")
            nc.vector.tensor_tensor(xm[:], xt[:], m0_t[:], ALU.mult)
            st = small.tile([P, 1], f32, tag="st")
            nc.vector.tensor_reduce(st[:], xm[:], axis=AX.X, op=ALU.add)
            nc.scalar.activation(ldcols[:, t:t + 1], st[:], ACTF.Ln)
            lp = lwork.tile([P, P], f32, tag="lp")
            nc.vector.tensor_tensor(lp[:], lt[:], pm_t[:], ALU.mult)
            nc.vector.tensor_reduce(t1cols[:, t:t + 1], lp[:], axis=AX.X,
                                    op=ALU.add)

        for t in range(T):
            d2 = tile_gram(t)
            tile_softmax(t, d2)

        # ---- final weighted reduction ----
        ld = ldcols
        z1 = work.tile([P, T], f32, tag="z1")
        nc.vector.tensor_tensor(z1[:], t1cols[:], w1_t[:], ALU.mult)
        z2 = work.tile([P, T], f32, tag="z2")
        nc.vector.tensor_tensor(z2[:], ld[:], w2_t[:], ALU.mult)
        zs = work.tile([P, T], f32, tag="zs")
        nc.vector.tensor_add(zs[:], z1[:], z2[:])
        zc = work.tile([P, 1], f32, tag="zc")
        nc.vector.tensor_reduce(zc[:], zs[:], axis=AX.X, op=ALU.add)

        ones = work.tile([P, 1], f32, tag="ones")
        nc.vector.memset(ones[:], 1.0)
        tot_ps = gpsum.tile([1, 1], f32, tag="g")
        nc.tensor.matmul(tot_ps[:, :], zc[:], ones[:], start=True, stop=True)
        tot = work.tile([1, 1], f32, tag="tot")
        nc.vector.tensor_copy(tot[:], tot_ps[:, :])
        nc.sync.dma_start(out_dram[:, :], tot[:])

    nc.compile()
    return nc


def _ensure_axon_hooks():
    """Provide antenv.axon_hooks if the image lacks it (NTFF profiling shim).

    Mirrors trn_agent_boot.trn_boot: the hook drives NRT profiling via the
    libaxon_pjrt.so C ABI.  If anything is missing we register a None hook,
    which makes bass_utils skip tracing gracefully instead of crashing.
    """
    try:
        import antenv.axon_hooks  # noqa: F401
        return
    except ImportError:
        pass
    import contextlib
    import ctypes
    import types

    import antenv

    hook = None
    so_path = "/opt/axon/libaxon_pjrt.so"
    try:
        lib = ctypes.CDLL(so_path)
        if hasattr(lib, "axon_start_nrt_profile"):
            lib.axon_start_nrt_profile.argtypes = [
                ctypes.POINTER(ctypes.c_int64), ctypes.c_size_t]
            lib.axon_start_nrt_profile.restype = ctypes.c_int64
            lib.axon_stop_nrt_profile.argtypes = [ctypes.c_char_p]
            lib.axon_stop_nrt_profile.restype = ctypes.c_int64

            @contextlib.contextmanager
            def _hook(output_dir, device_ids):
                import jax
                jax.devices()
                if device_ids:
                    ids = (ctypes.c_int64 * len(device_ids))(*device_ids)
                    rc = lib.axon_start_nrt_profile(ids, len(device_ids))
                else:
                    rc = lib.axon_start_nrt_profile(None, 0)
                if rc != 0:
                    raise RuntimeError(f"axon_start_nrt_profile rc={rc}")
                try:
                    yield
                finally:
                    n = lib.axon_stop_nrt_profile(str(output_dir).encode())
                    print(f"profile: {n} file(s) written to {output_dir}",
                          file=sys.stderr)

            hook = _hook
    except OSError:
        pass

    mod = types.ModuleType("antenv.axon_hooks")
    state = {"hook": hook}
    mod.get_axon_ntff_profile_hook = lambda: state["hook"]
    mod.set_axon_ntff_profile_hook = lambda h: state.__setitem__("hook", h)
    sys.modules["antenv.axon_hooks"] = mod
    antenv.axon_hooks = mod


def kernel(**inputs):
    global _compiled, LAST_RESULTS
    from concourse import bass_utils

    feats = np.ascontiguousarray(
        np.asarray(inputs["features"], dtype=np.float32).reshape(B * M, D))

    if _compiled is None:
        _compiled = (_build(), _host_consts())
    nc, consts = _compiled

    in_maps = []
    for k in range(NCORES):
        im = dict(consts)
        im["f"] = feats[k * ROWS:(k + 1) * ROWS]
        in_maps.append(im)

    trace = bool(os.environ.get("BASS_TRACE"))
    if trace:
        _ensure_axon_hooks()
    try:
        res = bass_utils.run_bass_kernel_spmd(
            nc, in_maps, core_ids=list(range(NCORES)), trace=trace)
    except Exception:
        if not trace:
            raise
        # Tracing plumbing failed; rerun untraced so the result is still valid.
        os.environ["BASS_NEVER_TRACE"] = "1"
        try:
            res = bass_utils.run_bass_kernel_spmd(
                nc, in_maps, core_ids=list(range(NCORES)), trace=False)
        finally:
            del os.environ["BASS_NEVER_TRACE"]
    LAST_RESULTS = res
    total = float(np.sum([np.float64(r["out"][0, 0]) for r in res.results]))
    return np.array(total, dtype=np.float32)


# revision 24
# speedup vs baseline: 1.0044x; 1.0044x over previous
"""Trainium2 Bass kernel for nn_BRCLoss (supervised-contrastive style loss).

Math (per batch sample b, matching the jax reference):
    f = features[b].reshape(24, 4096); fhat = f / ||f||_row
    logits = (fhat @ fhat.T) / 0.1                       # [24, 24]
    exp_logits = exp(logits) * (1 - I)
    log_prob = logits - log(exp_logits.sum(-1))
    mlpp = (mask * log_prob).sum(-1) / (mask.sum(-1) + 1e-6)
    loss = sum_b mean_m(-0.1 * mlpp) / 512               # scalar

`outputs` / `targets` are unused by the reference; only `features`
[512, 2, 12, 4096] f32 matters.  Pure data parallel: 64 samples per core,
per-core partial sums added on the host.

Per-core kernel (~90 us, HBM-read roofline ~70 us at ~360 GB/s):
  - 13 tiles of [120 rows, 4096] (5 samples each; the last tile re-reads the
    trailing 120 rows and the duplicated sample is zero-weighted via W1/W2).
  - Feature loads are SWDGE (gpsimd) DMAs that cast f32 -> bf16 in flight:
    HBM still reads the full f32 stream (the memory roofline), but SBUF
    writes halve, which relieves the port bottleneck shared with the
    sibling NeuronCore under 8-core SPMD.
  - Per tile: PE-transposes 32 bf16 chunks [120,128] -> PSUM (4 per bank),
    DVE/ACT copies them to SBUF, then 32 accumulating [128,120]^T[128,120]
    bf16 matmuls build the raw block-diagonal Gram G [120,120] (f32 PSUM).
  - Per-tile epilogue: d2 = 0.1*diag(G) via a pre-scaled identity mask;
    rnx = d2^-0.5 computed as exp(-0.5*ln(d2)); logits via
    L = diag(rnx) @ G @ diag(rnx), where the column scale runs on the PE as
    G @ diag(rnx) (G symmetric, so lhsT=G) and the row scale as a
    per-partition tensor_scalar; then exp, masked reductions, and ln of the
    denominator.  Host-precomputed masks/weights (m0/pm/w1/w2) fold the
    self-exclusion, positives graph, positive counts + eps, anchor mean,
    1/B and tile-12 deduplication into one final weighted dot product.
  - All activations (Ln/Exp/Copy) are pinned to the single
    natural_log_exp_and_others ACT table set (see _OneActSetBacc) so the
    kernel pays exactly one ACT_TABLE_LOAD instead of ~1.3 us per Ln<->Exp
    alternation.
"""

import os
import sys

import numpy as np

if "/opt/trn_rl_repo" not in sys.path:
    sys.path.insert(0, "/opt/trn_rl_repo")

# Problem constants (hardcoded; kernel.py must be self-contained).
B = 512
NV = 2
NCLS = 12
D = 4096
M = NV * NCLS              # 24 anchor rows per sample
NCORES = 8
SPC = B // NCORES          # 64 samples per core
ROWS = SPC * M             # 1536 feature rows per core
P = 120                    # rows per tile (5 samples)
G5 = P // M                # samples per tile
T = 13                     # tiles per core (12 aligned + 1 overlapping tail)
CH = 128                   # contraction chunk (PE partition limit)
NCH = D // CH              # 32 chunks
QUAD = 4                   # transposed chunks packed per PSUM bank
NQ = NCH // QUAD
TEMP = 0.1
EPS_POS = 1e-6

_ROW_STARTS = [P * t for t in range(T - 1)] + [ROWS - P]

_compiled = None           # (nc, const_in_map)
LAST_RESULTS = None        # BassKernelResults of the most recent run


def _host_consts():
    """Masks/weights shared by every core (the per-core sample grid is identical)."""
    i = np.arange(NCLS)
    graph = (np.abs(i[:, None] - i[None, :]) <= 1).astype(np.float32)   # [12,12]
    eye24 = np.eye(M, dtype=np.float32)
    mask24 = np.tile(graph, (NV, NV)) * (1.0 - eye24)                    # positives
    blk = np.kron(np.eye(G5, dtype=np.float32), np.ones((M, M), np.float32))
    m0 = (blk * (1.0 - np.eye(P, dtype=np.float32))).astype(np.float32)  # denom mask
    pm = np.kron(np.eye(G5, dtype=np.float32), mask24).astype(np.float32)
    im = (TEMP * np.eye(P)).astype(np.float32)   # folds the 1/temperature scale
    msum = np.tile(mask24.sum(1), G5).astype(np.float64)                 # [120], 3 or 5
    alpha = -TEMP / ((msum + EPS_POS) * M * B)                           # per-row weight
    valid = np.ones((P, T), np.float64)
    valid[:M, T - 1] = 0.0   # last tile re-reads sample 59 -> zero weight
    w1 = (alpha[:, None] * valid).astype(np.float32)
    w2 = ((-alpha * msum)[:, None] * valid).astype(np.float32)
    return {"m0": m0, "pm": pm, "im": im, "w1": w1, "w2": w2}


def _build():
    from contextlib import ExitStack

    from concourse import bacc, bass, masks, mybir, tile

    f32 = mybir.dt.float32
    bf16 = mybir.dt.bfloat16
    AX = mybir.AxisListType
    ALU = mybir.AluOpType
    ACTF = mybir.ActivationFunctionType

    import bass_rust as _bass_rust
    from concourse.hw_specs import get_activation_tables

    class _OneActSetBacc(bacc.Bacc):
        """Every activation used here (Ln, Exp, Copy) lives in the
        natural_log_exp_and_others ACT table set; restricting the set-choice
        pass to it means one ACT_TABLE_LOAD for the whole kernel instead of
        ~1.3us switches between the exp- and ln-anchored sets per use.  Other
        sets keep their list position (ids are positional) but advertise no
        functions, so the pass cannot pick them."""

        def insert_act_table_loads(self):
            has_activation = any(
                isinstance(i, mybir.InstActivation)
                for b in self.main_func.blocks
                for i in b.instructions
            )
            if not has_activation:
                return
            tables = [
                (n, (s if n == "natural_log_exp_and_others" else set()))
                for n, s in get_activation_tables(self.m.arch).items()
            ]
            _bass_rust.insert_act_table_loads(self, tables)

    nc = _OneActSetBacc("TRN2", target_bir_lowering=False, debug=False,
                        num_devices=NCORES)

    f_dram = nc.dram_tensor("f", (ROWS, D), f32, kind="ExternalInput")
    m0_dram = nc.dram_tensor("m0", (P, P), f32, kind="ExternalInput")
    pm_dram = nc.dram_tensor("pm", (P, P), f32, kind="ExternalInput")
    im_dram = nc.dram_tensor("im", (P, P), f32, kind="ExternalInput")
    w1_dram = nc.dram_tensor("w1", (P, T), f32, kind="ExternalInput")
    w2_dram = nc.dram_tensor("w2", (P, T), f32, kind="ExternalInput")
    out_dram = nc.dram_tensor("out", (1, 1), f32, kind="ExternalOutput")

    DSPLIT = 2                 # DMAs per feature tile
    DCOLS = D // DSPLIT

    with ExitStack() as ctx:
        tc = ctx.enter_context(tile.TileContext(nc))
        consts = ctx.enter_context(tc.tile_pool(name="consts", bufs=1))
        fpool = ctx.enter_context(tc.tile_pool(name="fpool", bufs=8))
        tcpool = ctx.enter_context(tc.tile_pool(name="tcpool", bufs=6))
        work = ctx.enter_context(tc.tile_pool(name="work", bufs=1))
        lwork = ctx.enter_context(tc.tile_pool(name="lwork", bufs=2))
        small = ctx.enter_context(tc.tile_pool(name="small", bufs=2))
        tpsum = ctx.enter_context(
            tc.tile_pool(name="tpsum", bufs=3, space=bass.MemorySpace.PSUM))
        gpsum = ctx.enter_context(
            tc.tile_pool(name="gpsum", bufs=2, space=bass.MemorySpace.PSUM))
        rpsum = ctx.enter_context(
            tc.tile_pool(name="rpsum", bufs=2, space=bass.MemorySpace.PSUM))

        # Feature loads lead the program so the sync-ring FIFO starts streaming
        # them immediately; constants ride the scalar HWDGE ring in parallel.
        def load_tile(ft, t):
            r0 = _ROW_STARTS[t]
            nsp = 8 if t == T - 1 else (4 if t == T - 2 else DSPLIT)   # finer tail chunks
            w = D // nsp
            for q in range(nsp):
                nc.gpsimd.dma_start(ft[:, q * w:(q + 1) * w],
                                    f_dram[r0:r0 + P, q * w:(q + 1) * w])

        ftiles = []
        for t in range(T):
            ft = fpool.tile([P, D], bf16, tag="f")
            if t < 3:
                load_tile(ft, t)
            ftiles.append(ft)

        identb = consts.tile([128, 128], bf16, tag="identb")
        masks.make_identity(nc, identb[:])
        m0_t = consts.tile([P, P], f32, tag="m0")
        pm_t = consts.tile([P, P], f32, tag="pm")
        im_t = consts.tile([P, P], f32, tag="im")
        w1_t = consts.tile([P, T], f32, tag="w1")
        w2_t = consts.tile([P, T], f32, tag="w2")
        nc.scalar.dma_start(m0_t[:], m0_dram[:, :])
        nc.scalar.dma_start(pm_t[:], pm_dram[:, :])
        nc.scalar.dma_start(im_t[:], im_dram[:, :])
        nc.scalar.dma_start(w1_t[:], w1_dram[:, :])
        nc.scalar.dma_start(w2_t[:], w2_dram[:, :])

        # Preload the exp/ln activation table set while DMA streams.
        warm = consts.tile([1, 2], f32, tag="warm")
        nc.vector.memset(warm[:], 1.0)
        nc.scalar.activation(warm[:, 1:2], warm[:, 0:1], ACTF.Exp)

        t1cols = work.tile([P, T], f32, tag="t1cols")   # sum(mask*logits) per tile
        ldcols = work.tile([P, T], f32, tag="ldcols")   # log softmax denominators
        egpool = ctx.enter_context(tc.tile_pool(name="egpool", bufs=4))
        egs = {}

        def tile_gram(t):
            ft = ftiles[t]
            if t >= 3:
                load_tile(ft, t)
            g = gpsum.tile([P, P], f32, tag="g")
            for q in range(NQ):
                tp = tpsum.tile([128, QUAD * P], bf16, tag="tp")
                for j in range(QUAD):
                    c = q * QUAD + j
                    nc.tensor.transpose(
                        tp[:, j * P:(j + 1) * P],
                        ft[:, c * CH:(c + 1) * CH],
                        identb[:P, :P],
                    )
                tcs = tcpool.tile([128, QUAD * P], bf16, tag="tc")
                if q % 2 == 0:
                    nc.vector.tensor_copy(tcs[:], tp[:])
                else:
                    nc.scalar.copy(tcs[:], tp[:])
                for j in range(QUAD):
                    c = q * QUAD + j
                    sl = tcs[:, j * P:(j + 1) * P]
                    nc.tensor.matmul(g[:], sl, sl,
                                     start=(c == 0), stop=(c == NCH - 1))
            eg = egpool.tile([P, P], bf16, tag="eg")
            nc.vector.tensor_copy(eg[:], g[:])
            egs[t] = eg
            # d2 = 0.1 * diag(G)  (im_t is pre-scaled by TEMP)
            scr = lwork.tile([P, P], f32, tag="scr")
            nc.vector.tensor_tensor(scr[:], g[:], im_t[:], ALU.mult)
            d2 = small.tile([P, 1], f32, tag="d2")
            nc.vector.tensor_reduce(d2[:], scr[:], axis=AX.X, op=ALU.add)
            return d2

        def tile_softmax(t, d2):
            # rnx = (0.1*d2)^-0.5 via exp/ln (same ACT table set);
            # logits L = diag(rnx) @ G @ diag(rnx); the column scaling runs on
            # the PE as G @ diag(rnx) (G is symmetric so lhsT=G is G^T), the
            # row scaling as a per-partition tensor_scalar.
            eg = egs.pop(t)
            lnv = small.tile([P, 1], f32, tag="lnv")
            nc.scalar.activation(lnv[:], d2[:], ACTF.Ln)
            rnx = small.tile([P, 1], f32, tag="rnx")
            nc.scalar.activation(rnx[:], lnv[:], ACTF.Exp, scale=-0.5)
            drn = lwork.tile([P, P], bf16, tag="drn")
            nc.vector.tensor_scalar(drn[:], im_t[:], rnx[:], 1.0 / TEMP,
                                    op0=ALU.mult, op1=ALU.mult)
            h_ps = rpsum.tile([P, P], f32, tag="r")
            nc.tensor.matmul(h_ps[:], eg[:], drn[:], start=True, stop=True)
            lt = lwork.tile([P, P], f32, tag="lt")
            nc.vector.tensor_scalar_mul(lt[:], h_ps[:], rnx[:])
            xt = lwork.tile([P, P], f32, tag="xt")
            nc.scalar.activation(xt[:], lt[:], ACTF.Exp)
            xm = lwork.tile([P, P], f32, tag="xm")
            nc.vector.tensor_tensor(xm[:], xt[:], m0_t[:], ALU.mult)
            st = small.tile([P, 1], f32, tag="st")
            nc.vector.tensor_reduce(st[:], xm[:], axis=AX.X, op=ALU.add)
            nc.scalar.activation(ldcols[:, t:t + 1], st[:], ACTF.Ln)
            lp = lwork.tile([P, P], f32, tag="lp")
            nc.vector.tensor_tensor(lp[:], lt[:], pm_t[:], ALU.mult)
            nc.vector.tensor_reduce(t1cols[:, t:t + 1], lp[:], axis=AX.X,
                                    op=ALU.add)

        for t in range(T):
            d2 = tile_gram(t)
            tile_softmax(t, d2)

        # ---- final weighted reduction ----
        ld = ldcols
        z1 = work.tile([P, T], f32, tag="z1")
        nc.vector.tensor_tensor(z1[:], t1cols[:], w1_t[:], ALU.mult)
        z2 = work.tile([P, T], f32, tag="z2")
        nc.vector.tensor_tensor(z2[:], ld[:], w2_t[:], ALU.mult)
        zs = work.tile([P, T], f32, tag="zs")
        nc.vector.tensor_add(zs[:], z1[:], z2[:])
        zc = work.tile([P, 1], f32, tag="zc")
        nc.vector.tensor_reduce(zc[:], zs[:], axis=AX.X, op=ALU.add)

        ones = work.tile([P, 1], f32, tag="ones")
        nc.vector.memset(ones[:], 1.0)
        tot_ps = gpsum.tile([1, 1], f32, tag="g")
        nc.tensor.matmul(tot_ps[:, :], zc[:], ones[:], start=True, stop=True)
        tot = work.tile([1, 1], f32, tag="tot")
        nc.vector.tensor_copy(tot[:], tot_ps[:, :])
        nc.sync.dma_start(out_dram[:, :], tot[:])

    nc.compile()
    return nc


def _ensure_axon_hooks():
    """Provide antenv.axon_hooks if the image lacks it (NTFF profiling shim).

    Mirrors trn_agent_boot.trn_boot: the hook drives NRT profiling via the
    libaxon_pjrt.so C ABI.  If anything is missing we register a None hook,
    which makes bass_utils skip tracing gracefully instead of crashing.
    """
    try:
        import antenv.axon_hooks  # noqa: F401
        return
    except ImportError:
        pass
    import contextlib
    import ctypes
    import types

    import antenv

    hook = None
    so_path = "/opt/axon/libaxon_pjrt.so"
    try:
        lib = ctypes.CDLL(so_path)
        if hasattr(lib, "axon_start_nrt_profile"):
            lib.axon_start_nrt_profile.argtypes = [
                ctypes.POINTER(ctypes.c_int64), ctypes.c_size_t]
            lib.axon_start_nrt_profile.restype = ctypes.c_int64
            lib.axon_stop_nrt_profile.argtypes = [ctypes.c_char_p]
            lib.axon_stop_nrt_profile.restype = ctypes.c_int64

            @contextlib.contextmanager
            def _hook(output_dir, device_ids):
                import jax
                jax.devices()
                if device_ids:
                    ids = (ctypes.c_int64 * len(device_ids))(*device_ids)
                    rc = lib.axon_start_nrt_profile(ids, len(device_ids))
                else:
                    rc = lib.axon_start_nrt_profile(None, 0)
                if rc != 0:
                    raise RuntimeError(f"axon_start_nrt_profile rc={rc}")
                try:
                    yield
                finally:
                    n = lib.axon_stop_nrt_profile(str(output_dir).encode())
                    print(f"profile: {n} file(s) written to {output_dir}",
                          file=sys.stderr)

            hook = _hook
    except OSError:
        pass

    mod = types.ModuleType("antenv.axon_hooks")
    state = {"hook": hook}
    mod.get_axon_ntff_profile_hook = lambda: state["hook"]
    mod.set_axon_ntff_profile_hook = lambda h: state.__setitem__("hook", h)
    sys.modules["antenv.axon_hooks"] = mod
    antenv.axon_hooks = mod


def kernel(**inputs):
    global _compiled, LAST_RESULTS
    from concourse import bass_utils

    feats = np.ascontiguousarray(
        np.asarray(inputs["features"], dtype=np.float32).reshape(B * M, D))

    if _compiled is None:
        _compiled = (_build(), _host_consts())
    nc, consts = _compiled

    in_maps = []
    for k in range(NCORES):
        im = dict(consts)
        im["f"] = feats[k * ROWS:(k + 1) * ROWS]
        in_maps.append(im)

    trace = bool(os.environ.get("BASS_TRACE"))
    if trace:
        _ensure_axon_hooks()
    try:
        res = bass_utils.run_bass_kernel_spmd(
            nc, in_maps, core_ids=list(range(NCORES)), trace=trace)
    except Exception:
        # Tracing plumbing or a transient device hiccup; retry once untraced.
        os.environ["BASS_NEVER_TRACE"] = "1"
        try:
            res = bass_utils.run_bass_kernel_spmd(
                nc, in_maps, core_ids=list(range(NCORES)), trace=False)
        finally:
            del os.environ["BASS_NEVER_TRACE"]
    LAST_RESULTS = res
    total = float(np.sum([np.float64(r["out"][0, 0]) for r in res.results]))
    return np.array(total, dtype=np.float32)
